# revision 3
# baseline (speedup 1.0000x reference)
"""Multi-head causal attention (B=4, T=2048, D=1024, H=16) on 8 TRN2 NeuronCores.

Sharding: 8 cores = 4 batches x 2 head-halves. Core c handles batch c//2 and
heads [ (c%2)*8, (c%2)*8+8 ).  Each core computes its half of the attention
output and its partial output projection; the host sums the two partial
projections per batch.

Per-core device kernel (all matmul inputs bf16, fp32 PSUM accumulation):
  phase A: Q^T, K^T  [512ch x 2048t] and V (natural [2048t x 512ch], stored
           with a ones-column per head for the softmax denominator)
  phase B: per head-pair, per 512-query block, flash-style causal attention:
           S^T tiles [128k x 512q] -> exp -> mask (diag tiles) ->
           O^T accumulation via matmul with V_aug (65th row = softmax sum l)
           -> normalize by 1/l (broadcast via rank-1 PE matmul)
  phase C: partial output projection  out[t, 1024] += attn_half @ W_o_half
"""

import numpy as np
import ml_dtypes

import concourse.bass as bass
import concourse.mybir as mybir
import concourse.tile as tile
from concourse import bacc
from concourse import bass_utils

BF16 = mybir.dt.bfloat16
F32 = mybir.dt.float32
AF = mybir.ActivationFunctionType

B, T, D = 4, 2048, 1024
H, DK = 16, 64
HALF = 512            # channels per core (8 heads)
KB = D // 128         # 8 contraction blocks for projections
TB = T // 128         # 16 t/k blocks of 128
QB = T // 512         # 4 query blocks of 512
NPAIR = 4             # head pairs per core (2 heads = 128 channels)
SCALE = float(DK) ** -0.5

N_CORES = 8

_PROG = None  # (nc, names) cache — build/compile once per process


def _build_program():
    nc = bacc.Bacc("TRN2", target_bir_lowering=False, debug=False)

    xt_d = nc.dram_tensor("xt", [KB, 128, T], BF16, kind="ExternalInput")
    wqt_d = nc.dram_tensor("wqt", [KB, 128, HALF], BF16, kind="ExternalInput")
    wkt_d = nc.dram_tensor("wkt", [KB, 128, HALF], BF16, kind="ExternalInput")
    wvt_d = nc.dram_tensor("wvt", [KB, 128, HALF], BF16, kind="ExternalInput")
    wot_d = nc.dram_tensor("wot", [4, 128, D], BF16, kind="ExternalInput")
    mask_d = nc.dram_tensor("mask", [128, 4, 512], BF16, kind="ExternalInput")
    out_d = nc.dram_tensor("out", [TB, 128, D], F32, kind="ExternalOutput")

    with tile.TileContext(nc) as tc:
        with (
            tc.tile_pool(name="const", bufs=1) as const,
            tc.tile_pool(name="sb_pt", bufs=4) as sb_pt,
            tc.tile_pool(name="sb_norm", bufs=2) as sb_norm,
            tc.tile_pool(name="sb_out", bufs=2) as sb_out,
            tc.tile_pool(name="ps_mm", bufs=2, space="PSUM") as ps_mm,
            tc.tile_pool(name="ps_st", bufs=3, space="PSUM") as ps_st,
            tc.tile_pool(name="ps_ot", bufs=2, space="PSUM") as ps_ot,
            tc.tile_pool(name="ps_rb", bufs=1, space="PSUM") as ps_rb,
        ):
            xt_sb = const.tile([128, KB, T], BF16, tag="xt")
            wqt_sb = const.tile([128, KB, HALF], BF16, tag="wqt")
            wkt_sb = const.tile([128, KB, HALF], BF16, tag="wkt")
            wvt_sb = const.tile([128, KB, HALF], BF16, tag="wvt")
            wot_sb = const.tile([128, 4, D], BF16, tag="wot")
            mask_sb = const.tile([128, 4, 512], BF16, tag="mask")
            qt_sb = const.tile([128, NPAIR, T], BF16, tag="qt")
            kt_sb = const.tile([128, NPAIR, T], BF16, tag="kt")
            vaug_sb = const.tile([128, TB, 8 * 65], BF16, tag="vaug")
            otn_sb = const.tile([128, NPAIR, T], BF16, tag="otn")
            ones_sb = const.tile([1, 64], F32, tag="ones")

            # input DMAs (kb-interleaved so the first K-accumulations can
            # start before everything has landed)
            for kb in range(KB):
                nc.sync.dma_start(xt_sb[:, kb, :], xt_d.ap()[kb])
                nc.sync.dma_start(wqt_sb[:, kb, :], wqt_d.ap()[kb])
                nc.sync.dma_start(wkt_sb[:, kb, :], wkt_d.ap()[kb])
                nc.sync.dma_start(wvt_sb[:, kb, :], wvt_d.ap()[kb])
            for cb in range(4):
                nc.sync.dma_start(wot_sb[:, cb, :], wot_d.ap()[cb])
            nc.sync.dma_start(mask_sb[:], mask_d.ap())
            nc.vector.memset(ones_sb[:], 1.0)
            for h in range(8):  # ones column per head in V_aug
                nc.vector.memset(vaug_sb[:, :, h * 65 + 64 : h * 65 + 65], 1.0)

            # ---- phase A: projections ----
            for dst_sb, w_sb in ((qt_sb, wqt_sb), (kt_sb, wkt_sb)):
                for mb in range(4):          # channel block of 128
                    for nb in range(4):      # t block of 512
                        acc = ps_mm.tile([128, 512], F32, tag="acc")
                        for kb in range(KB):
                            nc.tensor.matmul(
                                acc[:],
                                w_sb[:, kb, mb * 128 : (mb + 1) * 128],
                                xt_sb[:, kb, nb * 512 : (nb + 1) * 512],
                                start=(kb == 0),
                                stop=(kb == KB - 1),
                            )
                        nc.scalar.copy(
                            dst_sb[:, mb, nb * 512 : (nb + 1) * 512], acc[:]
                        )
            for tb in range(TB):
                acc = ps_mm.tile([128, 512], F32, tag="acc")
                for kb in range(KB):
                    nc.tensor.matmul(
                        acc[:],
                        xt_sb[:, kb, tb * 128 : (tb + 1) * 128],
                        wvt_sb[:, kb, :],
                        start=(kb == 0),
                        stop=(kb == KB - 1),
                    )
                # scatter the 8 head slices into the ones-padded V_aug layout
                nc.scalar.copy(
                    vaug_sb[:, tb, :].rearrange("p (h c) -> p h c", c=65)[:, :, 0:64],
                    acc[:].rearrange("p (h c) -> p h c", c=64),
                )

            # ---- phase B: causal attention per head pair ----
            for pair in range(NPAIR):
                h0 = 2 * pair
                for qb in range(QB):
                    jmax = 4 * qb + 3
                    ot0 = ps_ot.tile([65, 512], F32, tag="ot")
                    ot1 = ps_ot.tile([65, 512], F32, tag="ot")
                    for j in range(jmax + 1):
                        st0 = ps_st.tile([128, 512], F32, tag="st")
                        st1 = ps_st.tile([128, 512], F32, tag="st")
                        # S^T[k,q] for the two heads — packed in the PE array
                        # via row groups 0-1 (partitions 0:64) and 2-3 (64:128)
                        nc.tensor.matmul(
                            st0[:],
                            kt_sb[0:64, pair, j * 128 : (j + 1) * 128],
                            qt_sb[0:64, pair, qb * 512 : (qb + 1) * 512],
                        )
                        nc.tensor.matmul(
                            st1[:],
                            kt_sb[64:128, pair, j * 128 : (j + 1) * 128],
                            qt_sb[64:128, pair, qb * 512 : (qb + 1) * 512],
                        )
                        pt0 = sb_pt.tile([128, 512], BF16, tag="pt")
                        pt1 = sb_pt.tile([128, 512], BF16, tag="pt")
                        nc.scalar.activation(pt0[:], st0[:], AF.Exp, scale=SCALE)
                        nc.scalar.activation(pt1[:], st1[:], AF.Exp, scale=SCALE)
                        d = j - 4 * qb
                        if d >= 0:  # diagonal tile: apply causal staircase mask
                            nc.vector.tensor_mul(pt0[:], pt0[:], mask_sb[:, d, :])
                            nc.vector.tensor_mul(pt1[:], pt1[:], mask_sb[:, d, :])
                        nc.tensor.matmul(
                            ot0[:],
                            vaug_sb[:, j, h0 * 65 : (h0 + 1) * 65],
                            pt0[:],
                            start=(j == 0),
                            stop=(j == jmax),
                        )
                        nc.tensor.matmul(
                            ot1[:],
                            vaug_sb[:, j, (h0 + 1) * 65 : (h0 + 2) * 65],
                            pt1[:],
                            start=(j == 0),
                            stop=(j == jmax),
                        )
                    # normalize: attn^T = O^T * (1/l) and store bf16
                    for hh, ot in ((0, ot0), (1, ot1)):
                        rec = sb_norm.tile([1, 512], F32, tag="rec")
                        nc.vector.reciprocal(rec[:], ot[64:65, :])
                        rb = ps_rb.tile([64, 512], F32, tag="rb")
                        nc.tensor.matmul(rb[:], ones_sb[:], rec[:])
                        rbs = sb_norm.tile([64, 512], F32, tag="rbs")
                        nc.vector.tensor_copy(rbs[:], rb[:])
                        nc.vector.tensor_mul(
                            otn_sb[
                                hh * 64 : (hh + 1) * 64,
                                pair,
                                qb * 512 : (qb + 1) * 512,
                            ],
                            ot[0:64, :],
                            rbs[:],
                        )

            # ---- phase C: partial output projection ----
            for tb in range(TB):
                outc = sb_out.tile([128, D], F32, tag="outc")
                for nb in range(2):
                    acc = ps_mm.tile([128, 512], F32, tag="acc")
                    for cb in range(4):
                        nc.tensor.matmul(
                            acc[:],
                            otn_sb[:, cb, tb * 128 : (tb + 1) * 128],
                            wot_sb[:, cb, nb * 512 : (nb + 1) * 512],
                            start=(cb == 0),
                            stop=(cb == 3),
                        )
                    nc.vector.tensor_copy(outc[:, nb * 512 : (nb + 1) * 512], acc[:])
                nc.sync.dma_start(out_d.ap()[tb], outc[:])

    nc.compile()
    return nc


def _prep_core_inputs(X, W_q, W_k, W_v, W_o, mask_host, c):
    b, half = c // 2, c % 2
    ch = slice(half * HALF, (half + 1) * HALF)
    bf = ml_dtypes.bfloat16
    xt = np.ascontiguousarray(X[b].T).reshape(KB, 128, T).astype(bf)
    wqt = np.ascontiguousarray(W_q[ch, :].T).reshape(KB, 128, HALF).astype(bf)
    wkt = np.ascontiguousarray(W_k[ch, :].T).reshape(KB, 128, HALF).astype(bf)
    wvt = np.ascontiguousarray(W_v[ch, :].T).reshape(KB, 128, HALF).astype(bf)
    wot = np.ascontiguousarray(W_o[:, ch].T).reshape(4, 128, D).astype(bf)
    return {
        "xt": xt, "wqt": wqt, "wkt": wkt, "wvt": wvt, "wot": wot,
        "mask": mask_host,
    }


def _make_mask():
    kp = np.arange(128)[:, None]
    qf = np.arange(512)[None, :]
    m = np.zeros((128, 4, 512), np.float32)
    for d in range(4):
        m[:, d, :] = (qf >= kp + d * 128).astype(np.float32)
    return m.astype(ml_dtypes.bfloat16)


def kernel(X, W_q, W_k, W_v, W_o):
    global _PROG
    X = np.asarray(X, dtype=np.float32)
    W_q = np.asarray(W_q, dtype=np.float32)
    W_k = np.asarray(W_k, dtype=np.float32)
    W_v = np.asarray(W_v, dtype=np.float32)
    W_o = np.asarray(W_o, dtype=np.float32)

    if _PROG is None:
        _PROG = _build_program()
    nc = _PROG

    mask_host = _make_mask()
    in_maps = [
        _prep_core_inputs(X, W_q, W_k, W_v, W_o, mask_host, c)
        for c in range(N_CORES)
    ]
    res = bass_utils.run_bass_kernel_spmd(nc, in_maps, core_ids=list(range(N_CORES)))

    out = np.empty((B, T, D), np.float32)
    for b in range(B):
        p0 = res.results[2 * b]["out"].reshape(T, D)
        p1 = res.results[2 * b + 1]["out"].reshape(T, D)
        out[b] = p0 + p1
    return out


# revision 4
# speedup vs baseline: 1.6176x; 1.6176x over previous
"""Multi-head causal attention (B=4, T=2048, D=1024, H=16) on 8 TRN2 NeuronCores.

Sharding: 8 cores = 4 batches x 2 head-halves. Core c handles batch c//2 and
heads [ (c%2)*8, (c%2)*8+8 ).  Each core computes its half of the attention
output and its partial output projection; the host sums the two partial
projections per batch.

Per-core device kernel (matmul inputs bf16, fp32 PSUM accumulation):
  phase A (per head-pair): Q^T, K^T [128ch x 2048t] slices; once: V (natural
        [2048t x 512ch] layout, stored with a ones-column per head so the
        PV matmul also produces the softmax denominator l)
  phase B (per head-pair, per 512-query block): causal flash attention:
        S^T tiles [128k x 1024(2 heads)] -> one exp -> staircase mask mul on
        diagonal tiles -> O^T accumulation in PSUM (65 rows: 64 out + l)
        -> fast PSUM release via ACT copy; 1/l via approx reciprocal +
        gpsimd partition-broadcast, off the critical path
  phase C: partial output projection out[t, 1024] = attn_half @ W_o_half

Emission order interleaves phase-A work of pair p+1 after phase B of pair p so
the TensorE always has fill work (keeps the HAM clock-gate warm).
"""

import numpy as np
import ml_dtypes

import concourse.bass as bass
import concourse.mybir as mybir
import concourse.tile as tile
from concourse import bacc
from concourse import bass_utils

BF16 = mybir.dt.bfloat16
F32 = mybir.dt.float32
AF = mybir.ActivationFunctionType

B, T, D = 4, 2048, 1024
H, DK = 16, 64
HALF = 512            # channels per core (8 heads)
KB = D // 128         # 8 contraction blocks for projections
TB = T // 128         # 16 t/k blocks of 128
QB = T // 512         # 4 query blocks of 512
NPAIR = 4             # head pairs per core (2 heads = 128 channels)
SCALE = float(DK) ** -0.5

N_CORES = 8

_PROG = None  # compiled program cache


def _build_program():
    nc = bacc.Bacc("TRN2", target_bir_lowering=False, debug=False)

    xt_d = nc.dram_tensor("xt", [KB, 128, T], BF16, kind="ExternalInput")
    wqt_d = nc.dram_tensor("wqt", [KB, 128, HALF], BF16, kind="ExternalInput")
    wkt_d = nc.dram_tensor("wkt", [KB, 128, HALF], BF16, kind="ExternalInput")
    wvt_d = nc.dram_tensor("wvt", [KB, 128, HALF], BF16, kind="ExternalInput")
    wot_d = nc.dram_tensor("wot", [4, 128, D], BF16, kind="ExternalInput")
    mask_d = nc.dram_tensor("mask", [128, 4, 1024], BF16, kind="ExternalInput")
    out_d = nc.dram_tensor("out", [TB, 128, D], F32, kind="ExternalOutput")

    with tile.TileContext(nc) as tc:
        with (
            tc.tile_pool(name="const", bufs=1) as const,
            tc.tile_pool(name="sb_pt", bufs=4) as sb_pt,
            tc.tile_pool(name="sb_otu", bufs=4) as sb_otu,
            tc.tile_pool(name="sb_lr", bufs=4) as sb_lr,
            tc.tile_pool(name="sb_rbr", bufs=4) as sb_rbr,
            tc.tile_pool(name="sb_sc", bufs=2) as sb_sc,
            tc.tile_pool(name="sb_out", bufs=2) as sb_out,
            tc.tile_pool(name="ps_st", bufs=2, space="PSUM") as ps_st,
            tc.tile_pool(name="ps_ot", bufs=2, space="PSUM") as ps_ot,
            tc.tile_pool(name="ps_acc", bufs=1, space="PSUM") as ps_acc,
        ):
            xt_sb = const.tile([128, KB, T], BF16, tag="xt")
            wqt_sb = const.tile([128, KB, HALF], BF16, tag="wqt")
            wkt_sb = const.tile([128, KB, HALF], BF16, tag="wkt")
            wvt_sb = const.tile([128, KB, HALF], BF16, tag="wvt")
            wot_sb = const.tile([128, 4, D], BF16, tag="wot")
            mask_sb = const.tile([128, 4, 1024], BF16, tag="mask")
            qt_sb = const.tile([128, NPAIR, T], BF16, tag="qt")
            kt_sb = const.tile([128, NPAIR, T], BF16, tag="kt")
            vaug_sb = const.tile([128, TB, 8 * 65], BF16, tag="vaug")
            otn_sb = const.tile([128, NPAIR, T], BF16, tag="otn")

            for kb in range(KB):
                nc.sync.dma_start(xt_sb[:, kb, :], xt_d.ap()[kb])
                nc.sync.dma_start(wqt_sb[:, kb, :], wqt_d.ap()[kb])
                nc.sync.dma_start(wkt_sb[:, kb, :], wkt_d.ap()[kb])
                nc.sync.dma_start(wvt_sb[:, kb, :], wvt_d.ap()[kb])
            for cb in range(4):
                nc.sync.dma_start(wot_sb[:, cb, :], wot_d.ap()[cb])
            nc.sync.dma_start(mask_sb[:], mask_d.ap())
            for h in range(8):  # ones column per head in V_aug
                nc.vector.memset(vaug_sb[:, :, h * 65 + 64 : h * 65 + 65], 1.0)

            def emit_qk_proj(pair):
                for dst_sb, w_sb in ((qt_sb, wqt_sb), (kt_sb, wkt_sb)):
                    for nbp in range(2):  # pairs of 512-t blocks
                        acc = ps_acc.tile([128, 1024], F32, tag="acc")
                        for kb in range(KB):
                            lhs = w_sb[:, kb, pair * 128 : (pair + 1) * 128]
                            nc.tensor.matmul(
                                acc[:, 0:512],
                                lhs,
                                xt_sb[:, kb, nbp * 1024 : nbp * 1024 + 512],
                                start=(kb == 0),
                                stop=(kb == KB - 1),
                            )
                            nc.tensor.matmul(
                                acc[:, 512:1024],
                                lhs,
                                xt_sb[:, kb, nbp * 1024 + 512 : (nbp + 1) * 1024],
                                start=(kb == 0),
                                stop=(kb == KB - 1),
                            )
                        nc.vector.tensor_copy(
                            dst_sb[:, pair, nbp * 1024 : (nbp + 1) * 1024], acc[:]
                        )

            def emit_v_proj():
                for tbp in range(8):  # pairs of 128-t blocks
                    acc = ps_acc.tile([128, 1024], F32, tag="acc")
                    for kb in range(KB):
                        nc.tensor.matmul(
                            acc[:, 0:512],
                            xt_sb[:, kb, (2 * tbp) * 128 : (2 * tbp + 1) * 128],
                            wvt_sb[:, kb, :],
                            start=(kb == 0),
                            stop=(kb == KB - 1),
                        )
                        nc.tensor.matmul(
                            acc[:, 512:1024],
                            xt_sb[:, kb, (2 * tbp + 1) * 128 : (2 * tbp + 2) * 128],
                            wvt_sb[:, kb, :],
                            start=(kb == 0),
                            stop=(kb == KB - 1),
                        )
                    nc.vector.tensor_copy(
                        vaug_sb[:, 2 * tbp : 2 * tbp + 2, :].rearrange(
                            "p a (h c) -> p a h c", c=65
                        )[:, :, :, 0:64],
                        acc[:].rearrange("p (a h c) -> p a h c", a=2, c=64),
                    )

            def emit_attention(pair):
                h0 = 2 * pair
                for qb in range(QB):
                    jmax = 4 * qb + 3
                    qsl = slice(qb * 512, (qb + 1) * 512)
                    ot0 = ps_ot.tile([65, 512], F32, tag="ot")
                    ot1 = ps_ot.tile([65, 512], F32, tag="ot")
                    for j in range(jmax + 1):
                        jsl = slice(j * 128, (j + 1) * 128)
                        st = ps_st.tile([128, 1024], F32, tag="st")
                        nc.tensor.matmul(
                            st[:, 0:512], kt_sb[0:64, pair, jsl], qt_sb[0:64, pair, qsl]
                        )
                        nc.tensor.matmul(
                            st[:, 512:1024],
                            kt_sb[64:128, pair, jsl],
                            qt_sb[64:128, pair, qsl],
                        )
                        pt = sb_pt.tile([128, 1024], BF16, tag="pt")
                        nc.scalar.activation(pt[:], st[:], AF.Exp, scale=SCALE)
                        d = j - 4 * qb
                        if d >= 0:  # diagonal tile
                            nc.vector.tensor_mul(pt[:], pt[:], mask_sb[:, d, :])
                        nc.tensor.matmul(
                            ot0[:],
                            vaug_sb[:, j, h0 * 65 : (h0 + 1) * 65],
                            pt[:, 0:512],
                            start=(j == 0),
                            stop=(j == jmax),
                        )
                        nc.tensor.matmul(
                            ot1[:],
                            vaug_sb[:, j, (h0 + 1) * 65 : (h0 + 2) * 65],
                            pt[:, 512:1024],
                            start=(j == 0),
                            stop=(j == jmax),
                        )
                    for hh, ot in ((0, ot0), (1, ot1)):
                        otu = sb_otu.tile([64, 512], BF16, tag="otu")
                        nc.scalar.copy(otu[:], ot[0:64, :])
                        lrow = sb_lr.tile([1, 512], F32, tag="lrow")
                        nc.vector.tensor_copy(lrow[:], ot[64:65, :])
                        rec = sb_lr.tile([1, 512], F32, tag="rec")
                        nc.vector.reciprocal_approx_fast(rec[:], lrow[:])
                        rbr = sb_rbr.tile([64, 512], F32, tag="rbr")
                        nc.gpsimd.partition_broadcast(rbr[:], rec[0:1, :])
                        if hh == 0:
                            nc.vector.tensor_mul(
                                otn_sb[0:64, pair, qsl], otu[:], rbr[:]
                            )
                        else:
                            sc = sb_sc.tile([64, 512], BF16, tag="sc")
                            nc.vector.tensor_mul(sc[:], otu[:], rbr[:])
                            nc.sync.dma_start(otn_sb[64:128, pair, qsl], sc[:])

            # A(p0) -> B(p0) -> A(p1) -> B(p1) ... : phase-A matmuls of the
            # next pair fill TensorE gaps while ACT paces phase B
            for pair in range(NPAIR):
                emit_qk_proj(pair)
                if pair == 0:
                    emit_v_proj()
                emit_attention(pair)

            # ---- phase C: partial output projection ----
            for tb in range(TB):
                tsl = slice(tb * 128, (tb + 1) * 128)
                acc = ps_acc.tile([128, 1024], F32, tag="acc")
                for cb in range(4):
                    lhs = otn_sb[:, cb, tsl]
                    nc.tensor.matmul(
                        acc[:, 0:512],
                        lhs,
                        wot_sb[:, cb, 0:512],
                        start=(cb == 0),
                        stop=(cb == 3),
                    )
                    nc.tensor.matmul(
                        acc[:, 512:1024],
                        lhs,
                        wot_sb[:, cb, 512:1024],
                        start=(cb == 0),
                        stop=(cb == 3),
                    )
                outc = sb_out.tile([128, D], F32, tag="outc")
                nc.vector.tensor_copy(outc[:], acc[:])
                nc.sync.dma_start(out_d.ap()[tb], outc[:])

    nc.compile()
    return nc


def _prep_core_inputs(X, W_q, W_k, W_v, W_o, mask_host, c):
    b, half = c // 2, c % 2
    ch = slice(half * HALF, (half + 1) * HALF)
    bf = ml_dtypes.bfloat16
    xt = np.ascontiguousarray(X[b].T).reshape(KB, 128, T).astype(bf)
    wqt = np.ascontiguousarray(W_q[ch, :].T).reshape(KB, 128, HALF).astype(bf)
    wkt = np.ascontiguousarray(W_k[ch, :].T).reshape(KB, 128, HALF).astype(bf)
    wvt = np.ascontiguousarray(W_v[ch, :].T).reshape(KB, 128, HALF).astype(bf)
    wot = np.ascontiguousarray(W_o[:, ch].T).reshape(4, 128, D).astype(bf)
    return {
        "xt": xt, "wqt": wqt, "wkt": wkt, "wvt": wvt, "wot": wot,
        "mask": mask_host,
    }


def _make_mask():
    kp = np.arange(128)[:, None]
    qf = np.arange(512)[None, :]
    m = np.zeros((128, 4, 1024), np.float32)
    for d in range(4):
        keep = (qf >= kp + d * 128).astype(np.float32)
        m[:, d, 0:512] = keep
        m[:, d, 512:1024] = keep
    return m.astype(ml_dtypes.bfloat16)


def kernel(X, W_q, W_k, W_v, W_o):
    global _PROG
    X = np.asarray(X, dtype=np.float32)
    W_q = np.asarray(W_q, dtype=np.float32)
    W_k = np.asarray(W_k, dtype=np.float32)
    W_v = np.asarray(W_v, dtype=np.float32)
    W_o = np.asarray(W_o, dtype=np.float32)

    if _PROG is None:
        _PROG = _build_program()
    nc = _PROG

    mask_host = _make_mask()
    in_maps = [
        _prep_core_inputs(X, W_q, W_k, W_v, W_o, mask_host, c)
        for c in range(N_CORES)
    ]
    res = bass_utils.run_bass_kernel_spmd(nc, in_maps, core_ids=list(range(N_CORES)))

    out = np.empty((B, T, D), np.float32)
    for b in range(B):
        p0 = res.results[2 * b]["out"].reshape(T, D)
        p1 = res.results[2 * b + 1]["out"].reshape(T, D)
        out[b] = p0 + p1
    return out
